# revision 23
# baseline (speedup 1.0000x reference)
"""Trainium2 Bass kernel for nn_AttentionLayer (MLP-scored sparse attention).

Math (per batch row b):
  h1 = relu(q[b] @ W1[:E] + keys[b] @ W1[E:] + b1)     # [S, H]
  h2 = relu(h1 @ W2 + b2)                              # [S, G]
  s  = h2 @ W3 (+ b3: cancels in softmax)              # [S]
  w  = softmax(mask ? s : -inf) = exp(s)*mask / sum    # [S]
  attended = w @ keys[b]                               # [E]

Sharding: pure data-parallel, batch 4096 -> 512 per core across 8 cores.

Per-core strategy (batch group GRP=32, 16 groups, bf16 PE compute):
  - keys loaded [s-partition, b, e] fp32 (two s-blocks: 128 + 72 rows), cast
    to bf16 on gpsimd (natural layout also feeds the attended matvecs)
  - PE transpose-mode flips [s,e] tiles -> keys_T [e, b*200+s] for the MLP
  - W1: one bf16 matmul per 400-token chunk (2 batches), 2-way col packing;
    per-batch qW+b1 bias folded into the relu via per-partition tensor_scalar
    (stacked [h x 2, b] bias tile built by two 32-aligned PE matmuls)
  - W2: 4-way tile_position packing -> h2 stacked [128, 400]
  - W3: block-diag [128,4] stationary -> scores [4, 400] per 1600 tokens
  - scores reshaped to [b, s] via SBUF->SBUF DMA; softmax on DVE/ACT
  - attended: per-b matvec (w_T stationary [s,1], keys natural moving
    [s,128]), 4-way col-group packing into PSUM slots, strided DMA out

Transpose-mode matmuls only tolerate ONE semaphore wait (walrus S3_LW limit),
so each transpose phase is preceded by a tiny "joiner" matmul that reads the
phase's producer tiles — it absorbs the cross-engine waits and the following
transposes ride the PE vector clock.
"""

import os
import sys
import numpy as np

for _p in ("/opt/trn_rl_repo",):
    if _p not in sys.path and os.path.isdir(_p):
        sys.path.insert(0, _p)

B, S, E, H, G = 4096, 200, 128, 64, 32
NCORES = 8
BL = B // NCORES      # 512 batches per core
S0, S1 = 128, S - 128  # s-blocks: 128 + 72

_BUILD_CACHE = {}


def build_bass(bl=BL, grp=32, sim_safe=False):
    """Build the single-core Bass module (SPMD: same program on all cores)."""
    import concourse.bass as bass
    import concourse.tile as tile
    from concourse import mybir
    from concourse.masks import make_identity
    from contextlib import ExitStack

    f32 = mybir.dt.float32
    bf16 = mybir.dt.bfloat16
    i32 = mybir.dt.int32

    ngrp = bl // grp
    nchunk = grp // 2          # 400-token chunks per group (2 batches each)
    nhalf = nchunk // 2        # chunks per 64-partition half
    nstack = nchunk // 4       # W2/W3 stacks per group
    CW = 400                   # chunk width (tokens)
    hg = grp // 2

    nc = bass.Bass()

    query = nc.declare_dram_parameter("query", [bl, E], f32, isOutput=False)
    keys = nc.declare_dram_parameter("keys", [bl, S, E], f32, isOutput=False)
    mask = nc.declare_dram_parameter("mask", [bl, S], i32, isOutput=False)
    W1 = nc.declare_dram_parameter("W1", [2 * E, H], f32, isOutput=False)
    b1 = nc.declare_dram_parameter("b1", [H], f32, isOutput=False)
    W2 = nc.declare_dram_parameter("W2", [H, G], f32, isOutput=False)
    b2 = nc.declare_dram_parameter("b2", [G], f32, isOutput=False)
    W3 = nc.declare_dram_parameter("W3", [G, 1], f32, isOutput=False)
    att_out = nc.declare_dram_parameter("attended", [bl, E], f32, isOutput=True)
    w_out = nc.declare_dram_parameter("weights", [bl, S], f32, isOutput=True)

    with ExitStack() as ctx:
        tc = ctx.enter_context(tile.TileContext(nc))

        const = ctx.enter_context(tc.tile_pool(name="const", bufs=1))
        kio = ctx.enter_context(tc.tile_pool(name="kio", bufs=2))
        kbfp = ctx.enter_context(tc.tile_pool(name="kbfp", bufs=2))
        ktp = ctx.enter_context(tc.tile_pool(name="ktp", bufs=2))
        h1p = ctx.enter_context(tc.tile_pool(name="h1p", bufs=2))
        h2p = ctx.enter_context(tc.tile_pool(name="h2p", bufs=4))
        scp = ctx.enter_context(tc.tile_pool(name="scp", bufs=2))
        smp = ctx.enter_context(tc.tile_pool(name="smp", bufs=2))
        outp = ctx.enter_context(tc.tile_pool(name="outp", bufs=2))

        # PSUM pools (8 banks total)
        tp_ps = ctx.enter_context(tc.tile_pool(name="tp_ps", bufs=2, space="PSUM"))
        w1_ps = ctx.enter_context(tc.tile_pool(name="w1_ps", bufs=2, space="PSUM"))
        w2_ps = ctx.enter_context(tc.tile_pool(name="w2_ps", bufs=1, space="PSUM"))
        w3_ps = ctx.enter_context(tc.tile_pool(name="w3_ps", bufs=1, space="PSUM"))
        at_ps = ctx.enter_context(tc.tile_pool(name="at_ps", bufs=2, space="PSUM"))

        # ---- one-time constants ----
        ident = const.tile([128, 128], bf16)
        make_identity(nc, ident)
        ident_f = const.tile([128, 128], f32)
        make_identity(nc, ident_f)

        w1k_bf = const.tile([128, H], bf16)
        nc.gpsimd.dma_start(out=w1k_bf, in_=W1[E:2 * E, :])

        w2_2x = const.tile([128, G], bf16)
        nc.gpsimd.dma_start(
            out=w2_2x,
            in_=W2[:, :].rearrange("(r h) g -> r h g", r=1).broadcast_to([2, H, G]))

        w3_stage = const.tile([128, 1], f32)
        nc.gpsimd.dma_start(
            out=w3_stage,
            in_=W3[:, :].rearrange("(r g) o -> r g o", r=1).broadcast_to([4, G, 1]))
        mask4 = const.tile([128, 4], bf16)
        nc.vector.memset(mask4, 0.0)
        for q in range(4):
            nc.vector.memset(mask4[32 * q:32 * q + 32, q:q + 1], 1.0)
        w3bd = const.tile([128, 4], bf16)
        nc.vector.tensor_scalar_mul(w3bd, mask4, w3_stage[:, 0:1])

        b2_4x = const.tile([128, 1], f32)
        nc.gpsimd.dma_start(
            out=b2_4x,
            in_=b2[:].rearrange("(r g o) -> r g o", r=1, o=1).broadcast_to([4, G, 1]))

        b1_2x = const.tile([128, 1], f32)
        nc.gpsimd.dma_start(
            out=b1_2x,
            in_=b1[:].rearrange("(r h o) -> r h o", r=1, o=1).broadcast_to([2, H, 1]))

        w1q_sb = const.tile([128, H], f32)
        nc.gpsimd.dma_start(out=w1q_sb, in_=W1[0:E, :])

        # query natural tiles -> PE transposes -> qT [E, bl]
        pb = min(128, bl)
        q_nat = const.tile([pb, bl // pb, E], f32)
        nc.sync.dma_start(out=q_nat,
                          in_=query[:, :].rearrange("(t p) e -> p t e", p=pb))
        qT = const.tile([128, bl], f32)
        for t in range(bl // pb):
            q_ps = w1_ps.tile([128, pb], f32, tag="w1")
            if t == 0:
                # joiners: each ldweights absorbs ONE cross-engine wait so the
                # transpose-mode matmuls below carry at most one wait each
                # (walrus allows a single sync wait per Matmult/Ldweights)
                nc.tensor.ldweights(q_nat[0:1, 0, 0:2].bitcast(bf16))
                nc.tensor.ldweights(ident_f[0:1, 0:2].bitcast(bf16))
            nc.tensor.transpose(q_ps[:, 0:pb], q_nat[:, t, :], ident_f[0:pb, 0:pb])
            nc.vector.tensor_copy(out=qT[:, t * pb:(t + 1) * pb], in_=q_ps[:, 0:pb])

        for g in range(ngrp):
            g0 = g * grp

            # ---- load keys [s, b, e] (fp32) and cast to bf16 ----
            kn_f = kio.tile([128, grp, 2 * E], f32, tag="kn_f")
            nc.sync.dma_start(
                out=kn_f[:, :, 0:E],
                in_=keys[g0:g0 + grp, 0:S0, :].rearrange("b s e -> s b e"))
            nc.sync.dma_start(
                out=kn_f[0:S1, :, E:2 * E],
                in_=keys[g0:g0 + grp, S0:S, :].rearrange("b s e -> s b e"))

            kn_bf = kbfp.tile([128, grp, 2 * E], bf16, tag="kn_bf")
            nc.gpsimd.tensor_copy(out=kn_bf[:, :, 0:E], in_=kn_f[:, :, 0:E])
            nc.gpsimd.tensor_copy(out=kn_bf[0:S1, :, E:2 * E],
                                  in_=kn_f[0:S1, :, E:2 * E])

            mask_sb = smp.tile([grp, S], i32, tag="mask_i")
            nc.sync.dma_start(out=mask_sb, in_=mask[g0:g0 + grp, :])

            # ---- per-group qW+b1 bias, stacked [h x 2, b-half] ----
            nc.tensor.ldweights(w1q_sb[0:1, 0:2].bitcast(bf16))
            qw_ps = w3_ps.tile([128, hg], f32, tag="w3")
            nc.tensor.matmul(qw_ps[0:64, :], lhsT=w1q_sb,
                             rhs=qT[:, g0:g0 + hg],
                             start=True, stop=True, tile_position=(0, 0))
            nc.tensor.matmul(qw_ps[64:128, :], lhsT=w1q_sb,
                             rhs=qT[:, g0 + hg:g0 + grp],
                             start=True, stop=True, tile_position=(0, 64))
            qwbT2 = smp.tile([128, hg], f32, tag="qwbT2")
            nc.vector.tensor_scalar_add(out=qwbT2, in0=qw_ps, scalar1=b1_2x[:, 0:1])

            # ---- transpose keys -> keys_T [e, tok] (tok = b*200+s, b-major) ----
            keys_T = ktp.tile([128, grp * S], bf16, tag="keys_T")
            tc_last = {}
            for c in range(nchunk):
                t_ps = tp_ps.tile([128, CW], bf16, tag="tp")
                if c == 0:
                    nc.tensor.ldweights(kn_bf[0:1, 0, 0:1])  # absorb Pool(cast)
                if c >= 2:
                    # absorb the WAR wait on this psum buffer (c-2's copy)
                    nc.tensor.ldweights(keys_T[0:1, (c - 2) * CW:(c - 2) * CW + 1])
                for half in range(2):
                    b = 2 * c + half
                    nc.tensor.transpose(
                        t_ps[:, half * 200:half * 200 + 128],
                        kn_bf[:, b, 0:E], ident)
                    nc.tensor.transpose(
                        t_ps[:, half * 200 + 128:half * 200 + 200],
                        kn_bf[0:S1, b, E:2 * E], ident[0:S1, 0:S1])
                if c % 2 == 0:
                    tc_last['v'] = nc.vector.tensor_copy(
                        out=keys_T[:, c * CW:(c + 1) * CW], in_=t_ps)
                else:
                    tc_last['s'] = nc.scalar.copy(
                        out=keys_T[:, c * CW:(c + 1) * CW], in_=t_ps)

            # ---- W1 -> (+qW+b1, relu) -> h1 [64h x 2-stack, tok] bf16 ----
            h1 = h1p.tile([128, nhalf * CW], bf16, tag="h1")
            nc.tensor.ldweights(keys_T[0:1, (nchunk - 2) * CW:(nchunk - 2) * CW + 1])
            nc.tensor.ldweights(keys_T[0:1, (nchunk - 1) * CW:(nchunk - 1) * CW + 1])
            for c in range(nhalf):
                ps = w1_ps.tile([128, CW], f32, tag="w1")
                if c >= 2:
                    nc.tensor.ldweights(h1[0:1, (c - 2) * CW:(c - 2) * CW + 1])
                for half in range(2):
                    cc = c + half * nhalf
                    nc.tensor.matmul(
                        ps[64 * half:64 * half + 64, :], lhsT=w1k_bf,
                        rhs=keys_T[:, cc * CW:(cc + 1) * CW],
                        start=True, stop=True, tile_position=(0, 64 * half))
                for half in range(2):
                    col = c * CW + half * 200
                    bias = qwbT2[:, 2 * c + half:2 * c + half + 1]
                    if c % 2 == 0:
                        nc.vector.tensor_scalar(
                            out=h1[:, col:col + 200],
                            in0=ps[:, half * 200:half * 200 + 200],
                            scalar1=bias, scalar2=0.0,
                            op0=mybir.AluOpType.add, op1=mybir.AluOpType.max)
                    else:
                        nc.scalar.activation(
                            h1[:, col:col + 200], ps[:, half * 200:half * 200 + 200],
                            mybir.ActivationFunctionType.Relu, bias=bias)

            # ---- W2 (4-way packed) -> relu -> h2 ; W3 block-diag -> scores ----
            scores_sb = scp.tile([4, nstack, CW], f32, tag="scores")
            nc.tensor.ldweights(h1[0:1, (nhalf - 2) * CW:(nhalf - 2) * CW + 1])
            nc.tensor.ldweights(h1[0:1, (nhalf - 1) * CW:(nhalf - 1) * CW + 1])
            h2_prev = None
            for t in range(nstack):
                ps2 = w2_ps.tile([128, CW], f32, tag="w2")
                if h2_prev is not None:
                    nc.tensor.ldweights(h2_prev[0:1, 0:1])
                for q in range(4):
                    c = q * nstack + t
                    half = c // nhalf
                    cl = c % nhalf
                    nc.tensor.matmul(
                        ps2[32 * q:32 * q + 32, :],
                        lhsT=w2_2x[64 * half:64 * half + 64, :],
                        rhs=h1[64 * half:64 * half + 64, cl * CW:(cl + 1) * CW],
                        start=True, stop=True,
                        tile_position=(64 * half, 32 * q))
                h2 = h2p.tile([128, CW], bf16, tag="h2")
                nc.vector.tensor_scalar(
                    out=h2, in0=ps2, scalar1=b2_4x[:, 0:1], scalar2=0.0,
                    op0=mybir.AluOpType.add, op1=mybir.AluOpType.max)
                ps3 = w3_ps.tile([4, CW], f32, tag="w3")
                nc.tensor.matmul(ps3, lhsT=w3bd, rhs=h2, start=True, stop=True)
                nc.vector.tensor_copy(out=scores_sb[:, t, :], in_=ps3)
                h2_prev = h2

            # ---- scores [4, t, 400] -> [b, s] via sbuf->sbuf DMA ----
            # token (q, t, half, s): b = q*2*nstack + 2*t + half
            scores_bs = smp.tile([grp, S], f32, tag="scores_bs")
            nc.sync.dma_start(
                out=scores_bs,
                in_=scores_sb.rearrange("q t (h s) -> q t h s", h=2))

            # ---- softmax (no max-sub; masked exp) ----
            mask_f = smp.tile([grp, S], f32, tag="mask_f")
            nc.vector.tensor_copy(out=mask_f, in_=mask_sb)
            et = smp.tile([grp, S], f32, tag="et")
            nc.scalar.activation(et, scores_bs, mybir.ActivationFunctionType.Exp)
            em = smp.tile([grp, S], f32, tag="em")
            nc.vector.tensor_mul(em, et, mask_f)
            ssum = smp.tile([grp, 1], f32, tag="ssum")
            nc.vector.reduce_sum(out=ssum, in_=em, axis=mybir.AxisListType.X)
            rinv = smp.tile([grp, 1], f32, tag="rinv")
            nc.vector.reciprocal(rinv, ssum)
            w_f = smp.tile([grp, S], f32, tag="w_f")
            nc.vector.tensor_scalar_mul(w_f, em, rinv[:, 0:1])
            w_bf = smp.tile([grp, S], bf16, tag="w_bf")
            nc.vector.tensor_scalar_mul(w_bf, em, rinv[:, 0:1])
            nc.sync.dma_start(out=w_out[g0:g0 + grp, :], in_=w_f)

            # ---- w transposed for attended stationaries ----
            wt_ps = tp_ps.tile([128, 2 * grp], bf16, tag="tp")
            nc.tensor.ldweights(w_bf[0:1, 0:1])
            nc.tensor.transpose(wt_ps[:, 0:grp], w_bf[:, 0:S0],
                                ident[0:grp, 0:grp])
            nc.tensor.transpose(wt_ps[0:S1, grp:2 * grp], w_bf[:, S0:S],
                                ident[0:grp, 0:grp])
            wT = smp.tile([128, 2, grp], bf16, tag="wT")
            nc.vector.tensor_copy(out=wT[:, 0, :], in_=wt_ps[:, 0:grp])
            nc.vector.tensor_copy(out=wT[0:S1, 1, :],
                                  in_=wt_ps[0:S1, grp:2 * grp])

            # ---- attended: per-b matvec, packed into PSUM slots ----
            # b = j*8 + bank*4 + q -> psum[bank] partition 32j, cols 128q
            nc.tensor.ldweights(wT[0:1, 0, 0:1])
            nc.tensor.ldweights(wT[0:1, 1, 0:1])
            for bank in range(grp // 16):
                aps = at_ps.tile([128, 512], f32, tag="att")
                if sim_safe:
                    nc.vector.memset(aps, 0.0)
                for j in range(4):
                    for q in range(4):
                        b = j * 8 + bank * 4 + q
                        nc.tensor.matmul(
                            aps[32 * j:32 * j + 1, 128 * q:128 * q + 128],
                            lhsT=wT[:, 0, b:b + 1], rhs=kn_bf[:, b, 0:E],
                            start=True, stop=False, tile_position=(0, 32 * j))
                        nc.tensor.matmul(
                            aps[32 * j:32 * j + 1, 128 * q:128 * q + 128],
                            lhsT=wT[0:S1, 1, b:b + 1],
                            rhs=kn_bf[0:S1, b, E:2 * E],
                            start=False, stop=True, tile_position=(0, 32 * j))
                att_sb = outp.tile([128, 512], f32, tag="att_sb")
                nc.vector.tensor_copy(out=att_sb, in_=aps)
                nc.sync.dma_start(
                    out=att_out[g0:g0 + grp, :].rearrange(
                        "(j w q) e -> j w q e", j=4, w=2)[:, bank, :, :],
                    in_=att_sb.rearrange(
                        "(j r) (q e) -> j r q e", j=4, q=4)[:, 0, :, :])

    if not sim_safe:
        _split_matmul_waits(nc, mybir)
    return nc


def _split_matmul_waits(nc, mybir):
    """Walrus allows a single sync-wait per engine instruction; move extras
    onto preceding same-engine NoOps (identical queue semantics)."""
    k = 0
    for fn in nc.m.functions:
        for blk in fn.blocks:
            out = []
            for inst in blk.instructions:
                si = inst.sync_info
                if (si is not None and si.on_wait and len(si.on_wait) > 1
                        and getattr(inst, "engine", None) is not None):
                    waits = list(si.on_wait)
                    for w in waits[:-1]:
                        nop = mybir.InstNoOp(name=f"I-swsplit-{k}")
                        k += 1
                        nop.engine = inst.engine
                        nop.sync_info = mybir.SyncInfo(on_wait=[w], on_update=[])
                        out.append(nop)
                    inst.sync_info = mybir.SyncInfo(
                        on_wait=[waits[-1]], on_update=list(si.on_update))
                out.append(inst)
            blk.instructions = out


def _get_nc(bl=BL, grp=32):
    key = (bl, grp)
    if key not in _BUILD_CACHE:
        _BUILD_CACHE[key] = build_bass(bl, grp)
    return _BUILD_CACHE[key]


def kernel(query, keys, mask, W1, b1, W2, b2, W3, b3):
    """Full-input entry point: shards batch across 8 cores, runs SPMD, gathers."""
    from concourse.bass_utils import run_bass_kernel_spmd

    nc = _get_nc()
    query = np.ascontiguousarray(query, dtype=np.float32)
    keys = np.ascontiguousarray(keys, dtype=np.float32)
    mask_i = np.ascontiguousarray(mask, dtype=np.int32)
    reps = dict(
        W1=np.ascontiguousarray(W1, dtype=np.float32),
        b1=np.ascontiguousarray(b1, dtype=np.float32),
        W2=np.ascontiguousarray(W2, dtype=np.float32),
        b2=np.ascontiguousarray(b2, dtype=np.float32),
        W3=np.ascontiguousarray(W3, dtype=np.float32),
    )
    in_maps = []
    for c in range(NCORES):
        sl = slice(c * BL, (c + 1) * BL)
        in_maps.append(dict(query=query[sl], keys=keys[sl], mask=mask_i[sl], **reps))

    res = run_bass_kernel_spmd(nc, in_maps, core_ids=list(range(NCORES)))
    attended = np.concatenate([np.asarray(r["attended"]) for r in res.results], axis=0)
    weights = np.concatenate([np.asarray(r["weights"]) for r in res.results], axis=0)
    return attended, weights


if __name__ == "__main__":
    nc = build_bass()
    print("built ok")
